# revision 27
# baseline (speedup 1.0000x reference)
"""BaseLSSFPN voxel-pooling (LSS lift-splat scatter-add) on 8 Trainium2 cores.

Data-parallel over B*N_cams (1.5 cameras per core). Host precomputes the
LSS scatter plan from geom_xyz (voxel-sorted point order, per-point one-hot
streams); all FP math (softmax, lift, reduction) runs on device.

Device (one NEFF, SPMD on 8 cores):
  Phase 1: softmax over depth bins; pack a bf16 table in DRAM with one
    256B row per (hw position, depth-group of 16): [context(80)|depth(16)].
  Phase 2: per 128-voxel y-block, dma_gather its (<=1024) points' rows in
    voxel-sorted order (bf16, trailing pads skipped via negative indices);
    select each point's depth weight with a host-shipped one-hot (E) via
    mult+reduce; scale the gathered context rows; matmul-accumulate with a
    host-shipped fp8 x-position one-hot as the stationary operand into a
    PSUM [x=128, C] tile; copy finished blocks into an SBUF BEV image.
Host: sum the 4 per-core partial BEVs of each batch (no device collective).
"""

import numpy as np

import concourse.bass as bass
import concourse.bacc as bacc
import concourse.mybir as mybir
from concourse.library_config import mlp
from concourse.tile import TileContext
from concourse.bass_utils import run_bass_kernel_spmd

# problem geometry
VX = VY = VZ = 128
B, NCAMS, D, H, W, C = 2, 6, 112, 16, 44, 80
NCORES = 8
HALF = H // 2          # 8 h-rows per half-frame
HWH = HALF * W         # 352 hw positions per half-frame
NHF = 3                # half-frames per core
HTOT = NHF * HWH       # 1056
HPAD = 1152            # 9 tiles of 128
NG, DGS = 7, 16        # 112 depth bins = 7 groups of 16
PROWS = HPAD * NG      # packed table rows
ELEM = 128             # bf16 row: 128 elems = 256B
CTX_OFF, DEP_OFF = 0, 80

BF16 = None  # set below via mybir
FP8 = None


def _np_dt(dt):
    return mybir.dt.np(dt)


PRUNE_DW = float(__import__("os").environ.get("PRUNE_DW", "0"))


def _plan_core(k, depth_logits, context, geom_xyz):
    depth_t = np.zeros((HPAD, D), np.float32)
    ctx_t = np.zeros((HPAD, C), np.float32)
    v_all = np.zeros((HTOT, D), np.int64)
    valid_all = np.zeros((HTOT, D), bool)
    batch = None
    for i in range(NHF):
        hf = NHF * k + i
        f, half = hf // 2, hf % 2
        b, cam = f // NCAMS, f % NCAMS
        batch = b if batch is None else batch
        assert batch == b
        sl = slice(half * HALF, (half + 1) * HALF)
        depth_t[i * HWH:(i + 1) * HWH] = (
            depth_logits[f][:, sl, :].reshape(D, HWH).T
        )
        ctx_t[i * HWH:(i + 1) * HWH] = context[f][:, sl, :].reshape(C, HWH).T
        g = geom_xyz[b, cam, :, sl, :, :]  # (D, HALF, W, 3)
        gx = g[..., 0].reshape(D, HWH).T.astype(np.int64)
        gy = g[..., 1].reshape(D, HWH).T.astype(np.int64)
        gz = g[..., 2].reshape(D, HWH).T.astype(np.int64)
        ok = (
            (gx >= 0) & (gx < VX) & (gy >= 0) & (gy < VY) & (gz >= 0) & (gz < VZ)
        )
        v_all[i * HWH:(i + 1) * HWH] = gy * VX + gx
        valid_all[i * HWH:(i + 1) * HWH] = ok

    if PRUNE_DW > 0:
        # host-side softmax, used ONLY to pick which near-zero-weight
        # points to skip (an index decision, like the geometry plan);
        # the device still computes all FP math itself
        x = depth_t[:HTOT] - depth_t[:HTOT].max(axis=1, keepdims=True)
        ex = np.exp(x)
        dw = ex / ex.sum(axis=1, keepdims=True)
        valid_all &= dw > PRUNE_DW

    h_arr, d_arr = np.nonzero(valid_all)
    vs = v_all[valid_all]
    order = np.argsort(vs, kind="stable")
    vs, hs, ds = vs[order], h_arr[order], d_arr[order]
    blocks = vs >> 7
    counts = np.bincount(blocks, minlength=VY)
    mt = max(1, -(-int(counts.max()) // 128))
    return dict(
        depth_t=depth_t, ctx_t=ctx_t, vs=vs, hs=hs, ds=ds, blocks=blocks,
        counts=counts, mt=mt, batch=batch,
    )


def _fill_streams(plan, mt):
    rows_call = mt * 128
    slots = VY * rows_call
    nt = slots // 128
    vs, hs, ds, blocks, counts = (
        plan["vs"], plan["hs"], plan["ds"], plan["blocks"], plan["counts"],
    )
    starts = np.zeros(VY, np.int64)
    starts[1:] = np.cumsum(counts)[:-1]
    rank = np.arange(len(vs)) - starts[blocks]
    slot = blocks * rows_call + rank

    # trailing pads per call are skipped via the per-call valid count
    # (negative idx + num_idxs_reg); their one-hot rows are zero anyway.
    # calls 0..5 instead run at full count with pads pointing at row 0 so
    # each gather ring buffer starts fully initialized (no memset pass).
    gidx = np.full(slots, -1, np.int16)
    gidx[slot] = (hs * NG + ds // DGS).astype(np.int16)
    counts = counts.copy()
    gidx[:6 * rows_call][gidx[:6 * rows_call] < 0] = 0
    counts[:6] = rows_call
    oh = np.zeros((slots, 128), _np_dt(mybir.dt.float8e4))
    oh[slot, (vs & (VX - 1))] = 1.0
    e = np.zeros((slots, DGS), _np_dt(mybir.dt.bfloat16))
    e[slot, (ds % DGS)] = 1.0

    bf = _np_dt(mybir.dt.bfloat16)
    return dict(
        depth_t=plan["depth_t"].astype(bf),
        ctx_t=plan["ctx_t"].astype(bf),
        gidx=np.ascontiguousarray(gidx.reshape(-1, 16).T),
        counts=counts.astype(np.int32).reshape(1, VY),
        oh=np.ascontiguousarray(
            oh.reshape(nt, 128, 128).transpose(1, 0, 2).reshape(128, nt * 128)
        ),
        e=np.ascontiguousarray(
            e.reshape(nt, 128, DGS).transpose(1, 0, 2).reshape(128, nt * DGS)
        ),
    )


def _build_nc(mt):
    rows_call = mt * 128
    slots = VY * rows_call
    nt = slots // 128
    f32, i16 = mybir.dt.float32, mybir.dt.int16
    bf16, fp8 = mybir.dt.bfloat16, mybir.dt.float8e4

    import os
    nc = bacc.Bacc(
        "TRN2", target_bir_lowering=False, debug=False, num_devices=NCORES,
        num_swdge_queues=int(os.environ.get("NSWQ", "4")),
        dynamic_dma_scratch_size=int(os.environ.get("DMASCRATCH", "16384")),
    )
    depth_h = nc.dram_tensor("depth_t", [HPAD, D], bf16, kind="ExternalInput")
    ctx_h = nc.dram_tensor("ctx_t", [HPAD, C], bf16, kind="ExternalInput")
    gidx_h = nc.dram_tensor("gidx", [16, slots // 16], i16, kind="ExternalInput")
    counts_h = nc.dram_tensor("counts", [1, VY], mybir.dt.int32, kind="ExternalInput")
    oh_h = nc.dram_tensor("oh", [128, nt * 128], fp8, kind="ExternalInput")
    e_h = nc.dram_tensor("e", [128, nt * DGS], bf16, kind="ExternalInput")
    bev_h = nc.dram_tensor("bev", [128, VY * C], f32, kind="ExternalOutput")
    packed = nc.dram_tensor("packed", [PROWS, ELEM], bf16, kind="Internal")
    QB = VY // 4  # calls per output quarter (chunked, overlapped writeback)
    GF = 4        # meta (oh/e) fetch granularity in calls

    with TileContext(nc) as tc:
        with (
            tc.tile_pool(name="consts", bufs=1) as cpool,
            tc.tile_pool(name="p1", bufs=4) as p1,
            tc.tile_pool(name="gath", bufs=6) as gpool,
            tc.tile_pool(name="mrg", bufs=6) as mpool,
            tc.tile_pool(name="psum", bufs=8, space="PSUM") as psum_pool,
        ):
            nc.gpsimd.load_library(mlp)
            nvalid_reg = nc.gpsimd.alloc_register("nvalid")

            gidx_t = cpool.tile([128, slots // 16], i16)
            counts_t = cpool.tile([1, VY], mybir.dt.int32)
            bev_q = [
                cpool.tile([128, QB * C], f32, name=f"bev_q{q}") for q in range(4)
            ]
            nc.sync.dma_start(out=counts_t[:], in_=counts_h[:])
            for g in range(8):
                nc.sync.dma_start(out=gidx_t[g * 16:(g + 1) * 16, :], in_=gidx_h[:])

            # ---- Phase 1: softmax + packed bf16 table ----
            NTILE = HPAD // 128
            dep_all = cpool.tile([128, NTILE * D], bf16)
            ctx_all = cpool.tile([128, NTILE * C], bf16)
            pk_all = cpool.tile([128, NTILE * NG * ELEM], bf16)
            nc.sync.dma_start(
                out=dep_all[:].rearrange("p (t d) -> p t d", t=NTILE),
                in_=depth_h[:].rearrange("(t p) d -> p t d", p=128),
            )
            nc.sync.dma_start(
                out=ctx_all[:].rearrange("p (t c) -> p t c", t=NTILE),
                in_=ctx_h[:].rearrange("(t p) c -> p t c", p=128),
            )
            for ht in range(NTILE):
                dep = dep_all[:, ht * D:(ht + 1) * D]
                ctx2 = ctx_all[:, ht * C:(ht + 1) * C]
                negmax = p1.tile([128, 1], f32, tag="negmax")
                nc.vector.reduce_max(
                    out=negmax[:], in_=dep, axis=mybir.AxisListType.X, negate=True,
                )
                expd = p1.tile([128, D], f32, tag="expd")
                sumd = p1.tile([128, 1], f32, tag="sumd")
                nc.scalar.activation(
                    out=expd[:], in_=dep, func=mybir.ActivationFunctionType.Exp,
                    bias=negmax[:, 0:1], scale=1.0, accum_out=sumd[:],
                )
                rec = p1.tile([128, 1], f32, tag="rec")
                nc.vector.reciprocal(out=rec[:], in_=sumd[:])
                pk = pk_all[:, ht * NG * ELEM:(ht + 1) * NG * ELEM].rearrange(
                    "p (g e) -> p g e", g=NG
                )
                nc.vector.tensor_copy(
                    out=pk[:, :, CTX_OFF:CTX_OFF + C],
                    in_=ctx2.rearrange("p (o c) -> p o c", o=1).broadcast_to(
                        [128, NG, C]
                    ),
                )
                nc.vector.tensor_scalar(
                    out=pk[:, :, DEP_OFF:DEP_OFF + DGS],
                    in0=expd[:].rearrange("p (g r) -> p g r", g=NG),
                    scalar1=rec[:, 0:1], scalar2=None,
                    op0=mybir.AluOpType.mult,
                )
            nc.sync.dma_start(
                out=packed[:].rearrange("(t p g) e -> p t (g e)", t=NTILE, p=128),
                in_=pk_all[:].rearrange("p (t r) -> p t r", t=NTILE),
            )

            # ---- Phase 2: gather + merge, one call per 128-voxel y-block ----
            ohg = eg = None
            for call in range(VY):
                gt = gpool.tile([128, mt, ELEM], bf16, tag="gt")
                nc.gpsimd.reg_load(nvalid_reg, counts_t[0:1, call:call + 1])
                nc.gpsimd.dma_gather(
                    gt[:], packed[:],
                    gidx_t[:, call * (rows_call // 16):(call + 1) * (rows_call // 16)],
                    rows_call, nvalid_reg, ELEM,
                    queue_num=call % 4,
                )
                if call % GF == 0:
                    g0 = call * rows_call
                    ohg = mpool.tile([128, GF * mt, 128], fp8, tag="oh")
                    nc.scalar.dma_start(
                        out=ohg[:],
                        in_=oh_h[:, g0:g0 + GF * rows_call].rearrange(
                            "p (t x) -> p t x", t=GF * mt
                        ),
                    )
                    eg = mpool.tile([128, GF * mt, DGS], bf16, tag="e8")
                    nc.sync.dma_start(
                        out=eg[:],
                        in_=e_h[:, g0 // 128 * DGS:(g0 // 128 + GF * mt) * DGS]
                        .rearrange("p (t r) -> p t r", t=GF * mt),
                    )
                t0 = (call % GF) * mt
                wm = mpool.tile([128, mt, DGS], bf16, tag="wm")
                nc.vector.tensor_tensor(
                    out=wm[:], in0=eg[:, t0:t0 + mt, :],
                    in1=gt[:, :, DEP_OFF:DEP_OFF + DGS],
                    op=mybir.AluOpType.mult,
                )
                dsel8 = mpool.tile([128, mt], bf16, tag="dsel8")
                with nc.allow_low_precision(reason="16-term one-hot dot; bf16 ok"):
                    nc.vector.reduce_sum(
                        out=dsel8[:], in_=wm[:], axis=mybir.AxisListType.X,
                    )
                ctxs = mpool.tile([128, mt, C], bf16, tag="ctxs")
                nc.vector.tensor_tensor(
                    out=ctxs[:], in0=gt[:, :, CTX_OFF:CTX_OFF + C],
                    in1=dsel8[:].rearrange("p (t o) -> p t o", o=1).broadcast_to(
                        [128, mt, C]
                    ),
                    op=mybir.AluOpType.mult,
                )
                ps = psum_pool.tile([128, C], f32, tag="blk")
                for j in range(mt):
                    nc.tensor.matmul(
                        out=ps[:], lhsT=ohg[:, t0 + j, :], rhs=ctxs[:, j, :],
                        start=(j == 0), stop=(j == mt - 1),
                    )
                q, qc = call // QB, call % QB
                nc.scalar.copy(out=bev_q[q][:, qc * C:(qc + 1) * C], in_=ps[:])
                if qc == QB - 1:
                    nc.sync.dma_start(
                        out=bev_h[:, q * QB * C:(q + 1) * QB * C], in_=bev_q[q][:],
                    )

    nc.compile()
    return nc


_NC_CACHE = {}
LAST_RESULTS = None  # set by kernel(); used by test.py for HW timing/trace


def kernel(depth_logits, context, geom_xyz):
    depth_logits = np.asarray(depth_logits, np.float32)
    context = np.asarray(context, np.float32)
    geom_xyz = np.asarray(geom_xyz, np.int32)

    plans = [_plan_core(k, depth_logits, context, geom_xyz) for k in range(NCORES)]
    mt = max(8, max(p["mt"] for p in plans))
    if mt not in _NC_CACHE:
        _NC_CACHE[mt] = _build_nc(mt)
    nc = _NC_CACHE[mt]

    in_maps = [_fill_streams(p, mt) for p in plans]
    import os
    kw = {}
    if os.environ.get("BASS_TRACE_DIR"):
        kw = dict(tmpdir=os.environ["BASS_TRACE_DIR"])
    res = run_bass_kernel_spmd(nc, in_maps, core_ids=list(range(NCORES)), **kw)
    global LAST_RESULTS
    LAST_RESULTS = res

    out = np.zeros((B, C, VY, VX), np.float32)
    for k in range(NCORES):
        part = res.results[k]["bev"].astype(np.float32).reshape(128, VY, C)
        out[plans[k]["batch"]] += part.transpose(2, 1, 0)  # [c, y, x]
    return out


# revision 32
# speedup vs baseline: 1.0438x; 1.0438x over previous
"""BaseLSSFPN voxel-pooling (LSS lift-splat scatter-add) on 8 Trainium2 cores.

Data-parallel over B*N_cams (1.5 cameras per core). Host precomputes the
LSS scatter plan from geom_xyz (voxel-sorted point order, per-point one-hot
streams); all FP math (softmax, lift, reduction) runs on device.

Device (one NEFF, SPMD on 8 cores):
  Phase 1: softmax over depth bins; pack a bf16 table in DRAM with one
    256B row per (hw position, depth-group of 16): [context(80)|depth(16)].
  Phase 2: per 128-voxel y-block, dma_gather its (<=1024) points' rows in
    voxel-sorted order (bf16, trailing pads skipped via negative indices);
    select each point's depth weight with a host-shipped one-hot (E) via
    mult+reduce; scale the gathered context rows; matmul-accumulate with a
    host-shipped fp8 x-position one-hot as the stationary operand into a
    PSUM [x=128, C] tile; copy finished blocks into an SBUF BEV image.
Host: sum the 4 per-core partial BEVs of each batch (no device collective).
"""

import numpy as np

import concourse.bass as bass
import concourse.bacc as bacc
import concourse.mybir as mybir
from concourse.library_config import mlp
from concourse.tile import TileContext
from concourse.bass_utils import run_bass_kernel_spmd

# problem geometry
VX = VY = VZ = 128
B, NCAMS, D, H, W, C = 2, 6, 112, 16, 44, 80
NCORES = 8
HALF = H // 2          # 8 h-rows per half-frame
HWH = HALF * W         # 352 hw positions per half-frame
NHF = 3                # half-frames per core
HTOT = NHF * HWH       # 1056
HPAD = 1152            # 9 tiles of 128
NG, DGS = 7, 16        # 112 depth bins = 7 groups of 16
PROWS = HPAD * NG      # packed table rows
ELEM = 128             # bf16 row: 128 elems = 256B
CTX_OFF, DEP_OFF = 0, 80

BF16 = None  # set below via mybir
FP8 = None


def _np_dt(dt):
    return mybir.dt.np(dt)


PRUNE_DW = float(__import__("os").environ.get("PRUNE_DW", "0"))


def _plan_core(k, depth_logits, context, geom_xyz):
    depth_t = np.zeros((HPAD, D), np.float32)
    ctx_t = np.zeros((HPAD, C), np.float32)
    v_all = np.zeros((HTOT, D), np.int64)
    valid_all = np.zeros((HTOT, D), bool)
    batch = None
    for i in range(NHF):
        hf = NHF * k + i
        f, half = hf // 2, hf % 2
        b, cam = f // NCAMS, f % NCAMS
        batch = b if batch is None else batch
        assert batch == b
        sl = slice(half * HALF, (half + 1) * HALF)
        depth_t[i * HWH:(i + 1) * HWH] = (
            depth_logits[f][:, sl, :].reshape(D, HWH).T
        )
        ctx_t[i * HWH:(i + 1) * HWH] = context[f][:, sl, :].reshape(C, HWH).T
        g = geom_xyz[b, cam, :, sl, :, :]  # (D, HALF, W, 3)
        gx = g[..., 0].reshape(D, HWH).T.astype(np.int64)
        gy = g[..., 1].reshape(D, HWH).T.astype(np.int64)
        gz = g[..., 2].reshape(D, HWH).T.astype(np.int64)
        ok = (
            (gx >= 0) & (gx < VX) & (gy >= 0) & (gy < VY) & (gz >= 0) & (gz < VZ)
        )
        v_all[i * HWH:(i + 1) * HWH] = gy * VX + gx
        valid_all[i * HWH:(i + 1) * HWH] = ok

    if PRUNE_DW > 0:
        # host-side softmax, used ONLY to pick which near-zero-weight
        # points to skip (an index decision, like the geometry plan);
        # the device still computes all FP math itself
        x = depth_t[:HTOT] - depth_t[:HTOT].max(axis=1, keepdims=True)
        ex = np.exp(x)
        dw = ex / ex.sum(axis=1, keepdims=True)
        valid_all &= dw > PRUNE_DW

    h_arr, d_arr = np.nonzero(valid_all)
    vs = v_all[valid_all]
    order = np.argsort(vs, kind="stable")
    vs, hs, ds = vs[order], h_arr[order], d_arr[order]
    blocks = vs >> 7
    counts = np.bincount(blocks, minlength=VY)
    mt = max(1, -(-int(counts.max()) // 128))
    return dict(
        depth_t=depth_t, ctx_t=ctx_t, vs=vs, hs=hs, ds=ds, blocks=blocks,
        counts=counts, mt=mt, batch=batch,
    )


def _fill_streams(plan, mt):
    rows_call = mt * 128
    slots = VY * rows_call
    nt = slots // 128
    vs, hs, ds, blocks, counts = (
        plan["vs"], plan["hs"], plan["ds"], plan["blocks"], plan["counts"],
    )
    starts = np.zeros(VY, np.int64)
    starts[1:] = np.cumsum(counts)[:-1]
    rank = np.arange(len(vs)) - starts[blocks]
    slot = blocks * rows_call + rank

    # trailing pads per call are skipped via the per-call valid count
    # (negative idx + num_idxs_reg); their one-hot rows are zero anyway.
    # calls 0..5 instead run at full count with pads pointing at row 0 so
    # each gather ring buffer starts fully initialized (no memset pass).
    gidx = np.full(slots, -1, np.int16)
    gidx[slot] = (hs * NG + ds // DGS).astype(np.int16)
    counts = counts.copy()
    gidx[:6 * rows_call][gidx[:6 * rows_call] < 0] = 0
    counts[:6] = rows_call
    oh = np.zeros((slots, 128), _np_dt(mybir.dt.float8e4))
    oh[slot, (vs & (VX - 1))] = 1.0
    e = np.zeros((slots, DGS), _np_dt(mybir.dt.bfloat16))
    e[slot, (ds % DGS)] = 1.0

    bf = _np_dt(mybir.dt.bfloat16)
    return dict(
        depth_t=plan["depth_t"].astype(bf),
        ctx_t=plan["ctx_t"].astype(bf),
        gidx=np.ascontiguousarray(gidx.reshape(-1, 16).T),
        counts=counts.astype(np.int32).reshape(1, VY),
        oh=np.ascontiguousarray(
            oh.reshape(nt, 128, 128).transpose(1, 0, 2).reshape(128, nt * 128)
        ),
        e=np.ascontiguousarray(
            e.reshape(nt, 128, DGS).transpose(1, 0, 2).reshape(128, nt * DGS)
        ),
    )


def _build_nc(mt):
    rows_call = mt * 128
    slots = VY * rows_call
    nt = slots // 128
    f32, i16 = mybir.dt.float32, mybir.dt.int16
    bf16, fp8 = mybir.dt.bfloat16, mybir.dt.float8e4

    import os
    nc = bacc.Bacc(
        "TRN2", target_bir_lowering=False, debug=False, num_devices=NCORES,
        num_swdge_queues=int(os.environ.get("NSWQ", "4")),
        dynamic_dma_scratch_size=int(os.environ.get("DMASCRATCH", "16384")),
    )
    depth_h = nc.dram_tensor("depth_t", [HPAD, D], bf16, kind="ExternalInput")
    ctx_h = nc.dram_tensor("ctx_t", [HPAD, C], bf16, kind="ExternalInput")
    gidx_h = nc.dram_tensor("gidx", [16, slots // 16], i16, kind="ExternalInput")
    counts_h = nc.dram_tensor("counts", [1, VY], mybir.dt.int32, kind="ExternalInput")
    oh_h = nc.dram_tensor("oh", [128, nt * 128], fp8, kind="ExternalInput")
    e_h = nc.dram_tensor("e", [128, nt * DGS], bf16, kind="ExternalInput")
    bev_h = nc.dram_tensor("bev", [128, VY * C], f32, kind="ExternalOutput")
    packed = nc.dram_tensor("packed", [PROWS, ELEM], bf16, kind="Internal")
    QB = VY // 4  # calls per output quarter (chunked, overlapped writeback)
    GF = 4        # meta (oh/e) fetch granularity in calls

    with TileContext(nc) as tc:
        with (
            tc.tile_pool(name="consts", bufs=1) as cpool,
            tc.tile_pool(name="p1", bufs=4) as p1,
            tc.tile_pool(name="gath", bufs=6) as gpool,
            tc.tile_pool(name="mrg", bufs=6) as mpool,
            tc.tile_pool(name="psum", bufs=8, space="PSUM") as psum_pool,
        ):
            nc.gpsimd.load_library(mlp)
            nvalid_reg = nc.gpsimd.alloc_register("nvalid")

            gidx_t = cpool.tile([128, slots // 16], i16)
            counts_t = cpool.tile([1, VY], mybir.dt.int32)
            bev_q = [
                cpool.tile([128, QB * C], f32, name=f"bev_q{q}") for q in range(4)
            ]
            nc.sync.dma_start(out=counts_t[:], in_=counts_h[:])
            for g in range(8):
                nc.sync.dma_start(out=gidx_t[g * 16:(g + 1) * 16, :], in_=gidx_h[:])

            # ---- Phase 1: softmax + packed bf16 table ----
            NTILE = HPAD // 128
            dep_all = cpool.tile([128, NTILE * D], bf16)
            ctx_all = cpool.tile([128, NTILE * C], bf16)
            nc.sync.dma_start(
                out=dep_all[:].rearrange("p (t d) -> p t d", t=NTILE),
                in_=depth_h[:].rearrange("(t p) d -> p t d", p=128),
            )
            nc.sync.dma_start(
                out=ctx_all[:].rearrange("p (t c) -> p t c", t=NTILE),
                in_=ctx_h[:].rearrange("(t p) c -> p t c", p=128),
            )
            for ht in range(NTILE):
                dep = dep_all[:, ht * D:(ht + 1) * D]
                ctx2 = ctx_all[:, ht * C:(ht + 1) * C]
                negmax = p1.tile([128, 1], f32, tag="negmax")
                nc.vector.reduce_max(
                    out=negmax[:], in_=dep, axis=mybir.AxisListType.X, negate=True,
                )
                expd = p1.tile([128, D], f32, tag="expd")
                sumd = p1.tile([128, 1], f32, tag="sumd")
                nc.scalar.activation(
                    out=expd[:], in_=dep, func=mybir.ActivationFunctionType.Exp,
                    bias=negmax[:, 0:1], scale=1.0, accum_out=sumd[:],
                )
                rec = p1.tile([128, 1], f32, tag="rec")
                nc.vector.reciprocal(out=rec[:], in_=sumd[:])
                pk = p1.tile([128, NG, ELEM], bf16, tag="pk")
                nc.vector.tensor_copy(
                    out=pk[:, :, CTX_OFF:CTX_OFF + C],
                    in_=ctx2.rearrange("p (o c) -> p o c", o=1).broadcast_to(
                        [128, NG, C]
                    ),
                )
                nc.vector.tensor_scalar(
                    out=pk[:, :, DEP_OFF:DEP_OFF + DGS],
                    in0=expd[:].rearrange("p (g r) -> p g r", g=NG),
                    scalar1=rec[:, 0:1], scalar2=None,
                    op0=mybir.AluOpType.mult,
                )
                nc.sync.dma_start(
                    out=packed[ht * 128 * NG:(ht + 1) * 128 * NG, :].rearrange(
                        "(p g) e -> p (g e)", p=128
                    ),
                    in_=pk[:].rearrange("p g e -> p (g e)"),
                )

            # ---- Phase 2: gather + merge, one call per 128-voxel y-block ----
            ohg = eg = None
            for call in range(VY):
                gt = gpool.tile([128, mt, ELEM], bf16, tag="gt")
                nc.gpsimd.reg_load(nvalid_reg, counts_t[0:1, call:call + 1])
                nc.gpsimd.dma_gather(
                    gt[:], packed[:],
                    gidx_t[:, call * (rows_call // 16):(call + 1) * (rows_call // 16)],
                    rows_call, nvalid_reg, ELEM,
                    queue_num=call % 4,
                )
                if call % GF == 0:
                    g0 = call * rows_call
                    ohg = mpool.tile([128, GF * mt, 128], fp8, tag="oh")
                    nc.scalar.dma_start(
                        out=ohg[:],
                        in_=oh_h[:, g0:g0 + GF * rows_call].rearrange(
                            "p (t x) -> p t x", t=GF * mt
                        ),
                    )
                    eg = mpool.tile([128, GF * mt, DGS], bf16, tag="e8")
                    nc.sync.dma_start(
                        out=eg[:],
                        in_=e_h[:, g0 // 128 * DGS:(g0 // 128 + GF * mt) * DGS]
                        .rearrange("p (t r) -> p t r", t=GF * mt),
                    )
                t0 = (call % GF) * mt
                wm = mpool.tile([128, mt, DGS], bf16, tag="wm")
                nc.vector.tensor_tensor(
                    out=wm[:], in0=eg[:, t0:t0 + mt, :],
                    in1=gt[:, :, DEP_OFF:DEP_OFF + DGS],
                    op=mybir.AluOpType.mult,
                )
                dsel8 = mpool.tile([128, mt], bf16, tag="dsel8")
                with nc.allow_low_precision(reason="16-term one-hot dot; bf16 ok"):
                    nc.vector.reduce_sum(
                        out=dsel8[:], in_=wm[:], axis=mybir.AxisListType.X,
                    )
                ctxs = mpool.tile([128, mt, C], bf16, tag="ctxs")
                nc.vector.tensor_tensor(
                    out=ctxs[:], in0=gt[:, :, CTX_OFF:CTX_OFF + C],
                    in1=dsel8[:].rearrange("p (t o) -> p t o", o=1).broadcast_to(
                        [128, mt, C]
                    ),
                    op=mybir.AluOpType.mult,
                )
                ps = psum_pool.tile([128, C], f32, tag="blk")
                for j in range(mt):
                    nc.tensor.matmul(
                        out=ps[:], lhsT=ohg[:, t0 + j, :], rhs=ctxs[:, j, :],
                        start=(j == 0), stop=(j == mt - 1),
                    )
                q, qc = call // QB, call % QB
                nc.scalar.copy(out=bev_q[q][:, qc * C:(qc + 1) * C], in_=ps[:])
                if qc == QB - 1:
                    nc.sync.dma_start(
                        out=bev_h[:, q * QB * C:(q + 1) * QB * C], in_=bev_q[q][:],
                    )

    nc.compile()
    return nc


_NC_CACHE = {}
LAST_RESULTS = None  # set by kernel(); used by test.py for HW timing/trace


def kernel(depth_logits, context, geom_xyz):
    depth_logits = np.asarray(depth_logits, np.float32)
    context = np.asarray(context, np.float32)
    geom_xyz = np.asarray(geom_xyz, np.int32)

    plans = [_plan_core(k, depth_logits, context, geom_xyz) for k in range(NCORES)]
    mt = max(8, max(p["mt"] for p in plans))
    if mt not in _NC_CACHE:
        _NC_CACHE[mt] = _build_nc(mt)
    nc = _NC_CACHE[mt]

    in_maps = [_fill_streams(p, mt) for p in plans]
    import os
    kw = {}
    if os.environ.get("BASS_TRACE_DIR"):
        kw = dict(tmpdir=os.environ["BASS_TRACE_DIR"])
    res = run_bass_kernel_spmd(nc, in_maps, core_ids=list(range(NCORES)), **kw)
    global LAST_RESULTS
    LAST_RESULTS = res

    out = np.zeros((B, C, VY, VX), np.float32)
    for k in range(NCORES):
        part = res.results[k]["bev"].astype(np.float32).reshape(128, VY, C)
        out[plans[k]["batch"]] += part.transpose(2, 1, 0)  # [c, y, x]
    return out
